# revision 12
# baseline (speedup 1.0000x reference)
"""CLRNet loss kernel for Trainium2 (8 NeuronCores, data-parallel over batch).

Contract: kernel(predictions [3,512,192,78] f32, targets [512,4,78] f32,
seg_loss scalar f32) -> scalar f32 (full loss). Sharding: batch axis split
8 ways; each core computes partial (cls, reg, iou) sums over its 64 samples
x 3 stages; host combines.

Key algebra used on-device:
  ovr.sum  = 30*T - S_px,  union.sum = 30*T + S_px, where
  S_px[p,l] = sum_n mask[l,n] * |pred_px[p,n] - t_xs[l,n]| = 799 * S'
  S'[p,l]   = sum_n |max(t'[l,n] - p_hat[p,n], -1)| - n_invalid[l]
  (valid entries of t' = t/799 are in [0,1) so the clamp only hits invalid
   entries (t' ~ -125), each contributing exactly 1.0, subtracted exactly.)
The assignment distance is dist_px = S_px / (T + 1e-9); since scores are
ratios x/max(x), the 799 scale cancels and S' is used directly.
"""

import numpy as np

import concourse.bacc as bacc
import concourse.mybir as mybir
from concourse.tile import TileContext
from concourse.bass_utils import run_bass_kernel_spmd

F32 = mybir.dt.float32
Alu = mybir.AluOpType
AF = mybir.ActivationFunctionType
AX = mybir.AxisListType

STAGES, B, P, D = 3, 512, 192, 78
NPTS = 72
L = 4
NCORES = 8
BS = B // NCORES            # 64 batch samples per core
ROWS = STAGES * BS          # 192 virtual samples per core
IMG_W, IMG_H = 800.0, 320.0
N_STRIPS = float(NPTS - 1)
W_SCALE = IMG_W - 1.0       # 799
BIG = 1.0e30
PC = 64                     # prior chunk for the heavy S pipeline


def _build_block(nc, tc, pool, vpool, psum_pool, pd_small_dram, pd_xs_dram,
                 tg_dram, acc, s):
    """Emit one block of `s` samples (s <= 128 partitions)."""
    V, G, A, T = nc.vector, nc.gpsimd, nc.scalar, nc.tensor

    # ---------------- loads ----------------
    pd_s = pool.tile([s, P, 6], F32, tag="pd_small")
    nc.sync.dma_start(pd_s[:], pd_small_dram)
    tg = pool.tile([s, L, D], F32, tag="tg")
    r0 = 0
    for src in tg_dram:
        n = src.shape[0]
        nc.sync.dma_start(tg[r0:r0 + n], src)
        r0 += n
    pd_xs = pool.tile([s, P, NPTS], F32, tag="pd_xs")
    for pc in range(P // PC):
        nc.sync.dma_start(pd_xs[:, pc * PC:(pc + 1) * PC],
                          pd_xs_dram[:, pc * PC:(pc + 1) * PC])

    # big reusable [s, L, P] buffers
    bufs = [pool.tile([s, L, P], F32, tag=f"big{i}", name=f"big{i}") for i in range(8)]
    b0, b1, b2, b3, b4, b5, b6, b7 = bufs

    # ---------------- target-derived small tensors ----------------
    tp = pool.tile([s, L, NPTS], F32, tag="tp")          # t' = t_xs / 799
    V.tensor_scalar(tp[:], tg[:, :, 6:D], 1.0 / W_SCALE, None, op0=Alu.mult)
    validf = pool.tile([s, L], F32, tag="validf")
    V.tensor_scalar(validf[:], tg[:, :, 1], 1.0, None, op0=Alu.is_equal)
    invm = pool.tile([s, L, NPTS], F32, tag="invm")
    V.tensor_scalar(invm[:], tp[:], 0.0, None, op0=Alu.is_lt)
    n_inv = pool.tile([s, L], F32, tag="n_inv")
    V.tensor_reduce(out=n_inv[:], in_=invm[:], axis=AX.X, op=Alu.add)
    t_len = pool.tile([s, L], F32, tag="t_len")
    V.tensor_scalar(t_len[:], n_inv[:], -1.0, 72.0, op0=Alu.mult, op1=Alu.add)
    rec_tlen = pool.tile([s, L], F32, tag="rec_tlen")
    V.tensor_scalar(rec_tlen[:], t_len[:], 1e-9, None, op0=Alu.add)
    V.reciprocal(rec_tlen[:], rec_tlen[:])
    c1 = pool.tile([s, L], F32, tag="c1")                # 30*T/799
    V.tensor_scalar(c1[:], t_len[:], 30.0 / W_SCALE, None, op0=Alu.mult)
    c1eps = pool.tile([s, L], F32, tag="c1eps")
    V.tensor_scalar(c1eps[:], c1[:], 1e-9 / W_SCALE, None, op0=Alu.add)
    t_y = pool.tile([s, L], F32, tag="t_y")    # -(IMG_H-1)*tg2 (bias for dy)
    V.tensor_scalar(t_y[:], tg[:, :, 2], -(IMG_H - 1.0), None, op0=Alu.mult)
    ntx = pool.tile([s, L], F32, tag="ntx")    # -tg3 (bias for dx)
    V.tensor_scalar(ntx[:], tg[:, :, 3], -1.0, None, op0=Alu.mult)
    nth = pool.tile([s, L], F32, tag="nth")    # -tg4 (bias for th)
    V.tensor_scalar(nth[:], tg[:, :, 4], -1.0, None, op0=Alu.mult)
    # tstart = round(tg2*71) (exact integer recovery), tsum = tg5 + tstart
    tsum = pool.tile([s, L], F32, tag="tsum")
    ts0 = pool.tile([s, L], F32, tag="ts0")
    V.tensor_scalar(ts0[:], tg[:, :, 2], N_STRIPS, None, op0=Alu.mult)
    tsi = pool.tile([s, L], mybir.dt.int32, tag="tsi")
    V.tensor_copy(tsi[:], ts0[:])
    V.tensor_copy(tsum[:], tsi[:])          # round-to-nearest-even cast
    V.tensor_tensor(tsum[:], tsum[:], tg[:, :, 5], op=Alu.add)
    # reg targets g3[s,l,c]: [tg2*71, tg3, tg4*180]
    g3 = pool.tile([s, L, 3], F32, tag="g3")
    V.tensor_scalar(g3[:, :, 0], tg[:, :, 2], N_STRIPS, None, op0=Alu.mult)
    V.tensor_copy(g3[:, :, 1], tg[:, :, 3])
    V.tensor_scalar(g3[:, :, 2], tg[:, :, 4], 180.0, None, op0=Alu.mult)
    neg_pen = pool.tile([s, L], F32, tag="neg_pen")      # 0 valid / -BIG invalid
    V.tensor_scalar(neg_pen[:], validf[:], BIG, BIG, op0=Alu.mult, op1=Alu.subtract)
    iota4 = pool.tile([s, L], F32, tag="iota4")
    for j in range(L):
        V.memset(iota4[:, j:j + 1], float(j))
    eps12 = pool.tile([s, 1], F32, tag="eps12")
    V.memset(eps12[:], 1e-12)
    eps8 = pool.tile([s, 1], F32, tag="eps8")
    V.memset(eps8[:], 1e-8)
    c101 = pool.tile([s, 1], F32, tag="c101")
    V.memset(c101[:], 1.01)

    def bl(x):   # broadcast [s,L] -> [s,L,P] over priors
        return x.unsqueeze(2).to_broadcast((s, L, P))

    def bp(x):   # broadcast [s,P] -> [s,L,P] over lanes
        return x.unsqueeze(1).to_broadcast((s, L, P))

    # ---------------- S' [s,L,P] -> b0 ----------------
    Sp = b0
    nchunk = P // PC
    for l in range(L):
        for pc in range(nchunk):
            v = vpool.tile([s, PC, NPTS], F32, tag="v")
            # sub on GPS as plain TT (broadcast in0), clamp+reduce on DVE
            idx = l * nchunk + pc
            if idx % 2 == 0:
                G.tensor_tensor(
                    v[:], tp[:, l].unsqueeze(1).to_broadcast((s, PC, NPTS)),
                    pd_xs[:, pc * PC:(pc + 1) * PC], op=Alu.subtract)
                V.tensor_scalar_max(v[:], v[:], -1.0)
            else:
                V.scalar_tensor_tensor(
                    out=v[:], in0=pd_xs[:, pc * PC:(pc + 1) * PC], scalar=-1.0,
                    in1=tp[:, l].unsqueeze(1).to_broadcast((s, PC, NPTS)),
                    op0=Alu.mult, op1=Alu.add)
                G.tensor_scalar_max(v[:], v[:], -1.0)
            V.tensor_reduce(out=Sp[:, l, pc * PC:(pc + 1) * PC], in_=v[:],
                            axis=AX.X, op=Alu.add, apply_absolute_value=True)
    V.tensor_tensor(Sp[:], Sp[:], bl(n_inv[:]), op=Alu.subtract)

    # ---------------- assignment scores ----------------
    scr = b1

    def norm_score(out_t, x, tag):
        # out = 1.01 - x / max(masked max(x), 1e-30); x >= 0
        mx = pool.tile([s, 1], F32, tag=tag + "_mx")
        V.tensor_tensor(scr[:], x, bl(validf[:]), op=Alu.mult)
        V.tensor_reduce(out=mx[:], in_=scr[:], axis=AX.XY, op=Alu.max)
        V.tensor_scalar(mx[:], mx[:], 1e-30, -1.0, op0=Alu.max, op1=Alu.mult)
        V.reciprocal(mx[:], mx[:])  # = -1/max
        A.activation(out_t, x, AF.Identity, scale=mx[:], bias=c101[:])

    dist, ds = b2, b3
    V.tensor_tensor(dist[:], Sp[:], bl(rec_tlen[:]), op=Alu.mult)
    norm_score(ds[:], dist[:], "d")                      # b2 free
    dxy, sd = b4, b5
    for l in range(L):
        A.activation(sd[:, l], pd_s[:, :, 2], AF.Identity,
                     scale=IMG_H - 1.0, bias=t_y[:, l:l + 1])
        A.activation(dxy[:, l], pd_s[:, :, 3], AF.Identity,
                     scale=W_SCALE, bias=ntx[:, l:l + 1])
    V.tensor_tensor(sd[:], sd[:], sd[:], op=Alu.mult)     # dy^2
    V.tensor_tensor(dxy[:], dxy[:], dxy[:], op=Alu.mult)  # dx^2
    V.tensor_tensor(sd[:], sd[:], dxy[:], op=Alu.add)     # b4 free
    A.sqrt(sd[:], sd[:])
    ss = b6
    norm_score(ss[:], sd[:], "s")                         # b5 free
    th = b4
    for l in range(L):
        A.activation(th[:, l], pd_s[:, :, 4], AF.Identity,
                     scale=1.0, bias=nth[:, l:l + 1])
    A.activation(th[:], th[:], AF.Abs)
    ths = b5
    norm_score(ths[:], th[:], "t")                        # b4 free

    # cls cost (column 1 only): pos1 - neg1
    spc = pool.tile([s, P], F32, tag="spc")
    d01 = pool.tile([s, P], F32, tag="d01")
    V.scalar_tensor_tensor(out=d01[:], in0=pd_s[:, :, 0], scalar=-1.0,
                           in1=pd_s[:, :, 1], op0=Alu.mult, op1=Alu.add)
    A.activation(spc[:], pd_s[:, :, 1], AF.Sigmoid)
    lp = pool.tile([s, P], F32, tag="lp")
    A.activation(lp[:], spc[:], AF.Ln, bias=eps12[:])
    one_m = pool.tile([s, P], F32, tag="one_m")
    A.activation(one_m[:], spc[:], AF.Identity, scale=-1.0, bias=1.0)
    ln_ = pool.tile([s, P], F32, tag="ln_")
    A.activation(ln_[:], one_m[:], AF.Ln, bias=eps12[:])
    sq1 = pool.tile([s, P], F32, tag="sq1")
    A.activation(sq1[:], one_m[:], AF.Square)                  # (1-sp)^2
    sq0 = pool.tile([s, P], F32, tag="sq0")
    A.activation(sq0[:], spc[:], AF.Square)                    # sp^2
    clsc = pool.tile([s, P], F32, tag="clsc")
    V.scalar_tensor_tensor(out=clsc[:], in0=lp[:], scalar=-0.25, in1=sq1[:],
                           op0=Alu.mult, op1=Alu.mult)          # pos1
    V.scalar_tensor_tensor(out=ln_[:], in0=ln_[:], scalar=-0.75, in1=sq0[:],
                           op0=Alu.mult, op1=Alu.mult)          # neg1
    V.tensor_tensor(clsc[:], clsc[:], ln_[:], op=Alu.subtract)  # pos1 - neg1

    # ncost = 3*(ds*ss*ths)^2 - clsc, masked: *valid + neg_pen
    q = b2
    V.tensor_tensor(q[:], ds[:], ss[:], op=Alu.mult)
    V.tensor_tensor(q[:], q[:], ths[:], op=Alu.mult)      # b3,b5,b6 free
    ncost = b7
    V.scalar_tensor_tensor(out=ncost[:], in0=q[:], scalar=3.0, in1=q[:],
                           op0=Alu.mult, op1=Alu.mult)    # 3*q^2 ; b2 free
    V.tensor_tensor(ncost[:], ncost[:], bp(clsc[:]), op=Alu.subtract)
    V.tensor_tensor(ncost[:], ncost[:], bl(validf[:]), op=Alu.mult)
    V.tensor_tensor(ncost[:], ncost[:], bl(neg_pen[:]), op=Alu.add)

    # ---------------- iou [s,L,P] ----------------
    iou, den, rden = b3, b6, b5
    V.scalar_tensor_tensor(out=iou[:], in0=Sp[:], scalar=-1.0, in1=bl(c1[:]),
                           op0=Alu.mult, op1=Alu.add)     # c1 - S'
    V.tensor_tensor(den[:], Sp[:], bl(c1eps[:]), op=Alu.add)   # b0 free
    V.reciprocal_approx_accurate(rden[:], den[:], scratch=scr[:])  # b1,b6 free
    V.tensor_tensor(iou[:], iou[:], rden[:], op=Alu.mult)  # b5 free

    # ---------------- dynamic-k ----------------
    iw = b6
    V.scalar_tensor_tensor(out=iw[:], in0=iou[:], scalar=0.0, in1=bl(validf[:]),
                           op0=Alu.max, op1=Alu.mult)
    i8 = pool.tile([s, L, 8], F32, tag="i8")
    m8 = pool.tile([s, L, 8], F32, tag="m8")
    for l in range(L):
        V.max(out=i8[:, l], in_=iw[:, l])
        V.max(out=m8[:, l], in_=ncost[:, l])               # b6 free
    dks = pool.tile([s, L], F32, tag="dks")
    V.tensor_reduce(out=dks[:], in_=i8[:, :, 0:4], axis=AX.X, op=Alu.add)
    dkf = pool.tile([s, L], F32, tag="dkf")
    V.tensor_scalar(dkf[:], dks[:], 0.5, None, op0=Alu.subtract)
    dki = pool.tile([s, L], mybir.dt.int32, tag="dki")
    V.tensor_copy(dki[:], dkf[:])
    V.tensor_copy(dkf[:], dki[:])           # floor(dks) for non-integer dks
    V.tensor_scalar(dkf[:], dkf[:], 1.0, 0.0, op0=Alu.subtract, op1=Alu.max)
    V.tensor_scalar(dkf[:], dkf[:], 3.0, None, op0=Alu.min)     # dyn_k-1 in [0,3]
    eqm = pool.tile([s, L, L], F32, tag="eqm")
    V.tensor_tensor(eqm[:], iota4[:].unsqueeze(1).to_broadcast((s, L, L)),
                    dkf[:].unsqueeze(2).to_broadcast((s, L, L)), op=Alu.is_equal)
    V.tensor_tensor(eqm[:], eqm[:], m8[:, :, 0:4], op=Alu.mult)
    thr = pool.tile([s, L], F32, tag="thr")
    V.tensor_reduce(out=thr[:], in_=eqm[:], axis=AX.X, op=Alu.add)

    # M [s,L,P]
    M = b0
    V.tensor_tensor(M[:], ncost[:], bl(thr[:]), op=Alu.is_ge)
    V.tensor_tensor(M[:], M[:], bl(validf[:]), op=Alu.mult)
    nm_p = pool.tile([s, P], F32, tag="nm_p")
    V.tensor_reduce(out=nm_p[:], in_=M[:].rearrange("s l p -> s p l"),
                    axis=AX.X, op=Alu.add)
    multi = pool.tile([s, P], F32, tag="multi")
    V.tensor_scalar(multi[:], nm_p[:], 1.0, None, op0=Alu.is_gt)
    nmax_p = pool.tile([s, P], F32, tag="nmax_p")
    V.tensor_reduce(out=nmax_p[:], in_=ncost[:].rearrange("s l p -> s p l"),
                    axis=AX.X, op=Alu.max)
    oh = b4
    V.tensor_tensor(oh[:], ncost[:], bp(nmax_p[:]), op=Alu.is_equal)  # b7 free
    notmulti = pool.tile([s, P], F32, tag="notmulti")
    A.activation(notmulti[:], multi[:], AF.Identity, scale=-1.0, bias=1.0)
    V.tensor_tensor(M[:, 0], M[:, 0], notmulti[:], op=Alu.mult)
    V.tensor_tensor(oh[:], oh[:], bp(multi[:]), op=Alu.mult)
    V.tensor_tensor(M[:], M[:], oh[:], op=Alu.max)        # b4 free
    n_match = pool.tile([s, 1], F32, tag="n_match")
    V.tensor_reduce(out=n_match[:], in_=M[:], axis=AX.XY, op=Alu.add)

    # ---------------- cls term ----------------
    matched = pool.tile([s, P], F32, tag="matched")
    V.tensor_reduce(out=matched[:], in_=M[:].rearrange("s l p -> s p l"),
                    axis=AX.X, op=Alu.add)
    matchedu = pool.tile([s, P], mybir.dt.uint32, tag="matchedu")
    V.tensor_scalar(matchedu[:], matched[:], 0.0, None, op0=Alu.is_gt)
    p1e = pool.tile([s, P], F32, tag="p1e")
    A.activation(p1e[:], d01[:], AF.Sigmoid)
    A.activation(p1e[:], p1e[:], AF.Identity, bias=eps8[:])
    p0e = pool.tile([s, P], F32, tag="p0e")
    A.activation(p0e[:], d01[:], AF.Sigmoid, scale=-1.0)
    A.activation(p0e[:], p0e[:], AF.Identity, bias=eps8[:])
    l1t = pool.tile([s, P], F32, tag="l1t")
    A.activation(l1t[:], p1e[:], AF.Ln)
    l0t = pool.tile([s, P], F32, tag="l0t")
    A.activation(l0t[:], p0e[:], AF.Ln)
    A.activation(p1e[:], p1e[:], AF.Identity, scale=-1.0, bias=1.0)
    A.activation(p0e[:], p0e[:], AF.Identity, scale=-1.0, bias=1.0)
    A.activation(p1e[:], p1e[:], AF.Square)               # (1-p1)^2
    A.activation(p0e[:], p0e[:], AF.Square)               # (1-p0)^2
    f1 = pool.tile([s, P], F32, tag="f1")
    V.scalar_tensor_tensor(out=f1[:], in0=l1t[:], scalar=-0.25, in1=p1e[:],
                           op0=Alu.mult, op1=Alu.mult)
    f0 = pool.tile([s, P], F32, tag="f0")
    V.scalar_tensor_tensor(out=f0[:], in0=l0t[:], scalar=-0.25, in1=p0e[:],
                           op0=Alu.mult, op1=Alu.mult)
    V.copy_predicated(f0[:], matchedu[:], f1[:])   # f0 := where(matched, f1, f0)
    num_t = pool.tile([s, 1], F32, tag="num_t")
    V.tensor_reduce(out=num_t[:], in_=validf[:], axis=AX.X, op=Alu.add)
    V.tensor_scalar(num_t[:], num_t[:], 1.0, None, op0=Alu.max)
    V.reciprocal(num_t[:], num_t[:])
    trip = pool.tile([s, 3], F32, tag="trip")
    V.tensor_reduce(out=trip[:, 0:1], in_=f0[:], axis=AX.X, op=Alu.add)
    V.tensor_scalar(trip[:, 0:1], trip[:, 0:1], num_t[:], None, op0=Alu.mult)

    # ---------------- reg term ----------------
    yx = pool.tile([s, 4, P], F32, tag="yx")     # pred_yxtl, c-major
    scales = [N_STRIPS, W_SCALE, 180.0, N_STRIPS]
    for c in range(4):
        A.mul(yx[:, c], pd_s[:, :, 2 + c], scales[c])
    pstart = pool.tile([s, P], F32, tag="pstart")
    V.tensor_scalar(pstart[:], pd_s[:, :, 2], N_STRIPS, None, op0=Alu.mult)
    psi = pool.tile([s, P], mybir.dt.int32, tag="psi")
    V.tensor_copy(psi[:], pstart[:])
    V.tensor_copy(pstart[:], psi[:])        # jnp.round (RNE)
    V.tensor_scalar(pstart[:], pstart[:], 0.0, N_STRIPS, op0=Alu.max, op1=Alu.min)

    def smooth_l1(a, qq, cnd, shp):
        # in-place: a := where(|a|<1, 0.5*a^2, |a|-0.5); cnd is uint32
        A.activation(a, a, AF.Abs)
        V.scalar_tensor_tensor(out=qq, in0=a, scalar=0.5, in1=a,
                               op0=Alu.mult, op1=Alu.mult)
        V.tensor_scalar(cnd, a, 1.0, None, op0=Alu.is_lt)
        V.tensor_scalar(a, a, 0.5, None, op0=Alu.subtract)
        V.copy_predicated(a, cnd, qq)

    diff3 = pool.tile([s, L, 3, P], F32, tag="diff3")
    d3q = pool.tile([s, L, 3, P], F32, tag="d3q")
    d3c = pool.tile([s, L, 3, P], mybir.dt.uint32, tag="d3c")
    V.tensor_tensor(diff3[:], yx[:, 0:3].unsqueeze(1).to_broadcast((s, L, 3, P)),
                    g3[:].unsqueeze(3).to_broadcast((s, L, 3, P)), op=Alu.subtract)
    smooth_l1(diff3[:], d3q[:], d3c[:], None)
    slsum = b6
    V.tensor_reduce(out=slsum[:], in_=diff3[:].rearrange("s l c p -> s l p c"),
                    axis=AX.X, op=Alu.add)
    tlp = b4
    V.tensor_tensor(tlp[:], bl(tsum[:]), bp(pstart[:]), op=Alu.subtract)
    V.tensor_tensor(tlp[:], bp(yx[:, 3]), tlp[:], op=Alu.subtract)  # yxtl3 - tlp
    cndu = pool.tile([s, L, P], mybir.dt.uint32, tag="cndu")
    smooth_l1(tlp[:], b5[:], cndu[:], None)
    V.tensor_tensor(slsum[:], slsum[:], tlp[:], op=Alu.add)
    V.tensor_tensor(slsum[:], slsum[:], M[:], op=Alu.mult)
    rden4 = pool.tile([s, 1], F32, tag="rden4")
    V.tensor_scalar(rden4[:], n_match[:], 4.0, 1.0, op0=Alu.mult, op1=Alu.max)
    V.reciprocal(rden4[:], rden4[:])
    V.tensor_reduce(out=trip[:, 1:2], in_=slsum[:], axis=AX.XY, op=Alu.add)
    V.tensor_scalar(trip[:, 1:2], trip[:, 1:2], rden4[:], None, op0=Alu.mult)

    # ---------------- iou term ----------------
    A.activation(iou[:], iou[:], AF.Identity, scale=-1.0, bias=1.0)
    V.tensor_tensor(iou[:], iou[:], M[:], op=Alu.mult)
    rnm = pool.tile([s, 1], F32, tag="rnm")
    V.tensor_scalar(rnm[:], n_match[:], 1.0, None, op0=Alu.max)
    V.reciprocal(rnm[:], rnm[:])
    V.tensor_reduce(out=trip[:, 2:3], in_=iou[:], axis=AX.XY, op=Alu.add)
    V.tensor_scalar(trip[:, 2:3], trip[:, 2:3], rnm[:], None, op0=Alu.mult)

    # ---------------- cross-partition sum via PE ----------------
    ones = pool.tile([s, 1], F32, tag="ones")
    V.memset(ones[:], 1.0)
    part = psum_pool.tile([1, 3], F32, tag="psum_part")
    T.matmul(part[:], ones[:], trip[:], start=True, stop=True)
    V.tensor_tensor(acc[:1, 0:3], acc[:1, 0:3], part[:], op=Alu.add)


def build():
    nc = bacc.Bacc("TRN2", target_bir_lowering=False, debug=False)
    preds = nc.dram_tensor("preds", [ROWS, P * D], F32, kind="ExternalInput").ap()
    tgts = nc.dram_tensor("tgts", [BS, L * D], F32, kind="ExternalInput").ap()
    outp = nc.dram_tensor("out", [1, 4], F32, kind="ExternalOutput").ap()

    pd3 = preds.rearrange("r (p d) -> r p d", d=D)
    tg3 = tgts.rearrange("r (l d) -> r l d", d=D)

    with TileContext(nc) as tc:
        from contextlib import ExitStack
        with ExitStack() as ctx:
            pool = ctx.enter_context(tc.tile_pool(name="main", bufs=1))
            vpool = ctx.enter_context(tc.tile_pool(name="vp", bufs=3))
            psum_pool = ctx.enter_context(tc.tile_pool(name="ps", bufs=2, space="PSUM"))
            acc = pool.tile([1, 4], F32, tag="acc")
            nc.vector.memset(acc[:], 0.0)
            # block 0: stages 0,1 (rows 0..127); block 1: stage 2 (rows 128..191)
            _build_block(nc, tc, pool, vpool, psum_pool,
                         pd3[0:128, :, 0:6], pd3[0:128, :, 6:D],
                         [tg3, tg3], acc, 128)
            _build_block(nc, tc, pool, vpool, psum_pool,
                         pd3[128:192, :, 0:6], pd3[128:192, :, 6:D],
                         [tg3], acc, 64)
            nc.sync.dma_start(outp[:], acc[:])
    nc.compile()
    return nc


_NC_CACHE = None


def _get_nc():
    global _NC_CACHE
    if _NC_CACHE is None:
        _NC_CACHE = build()
    return _NC_CACHE


def kernel(predictions, targets, seg_loss):
    nc = _get_nc()
    in_maps = []
    for c in range(NCORES):
        sl = slice(BS * c, BS * (c + 1))
        p = np.ascontiguousarray(predictions[:, sl]).reshape(ROWS, P * D)
        t = np.ascontiguousarray(targets[sl]).reshape(BS, L * D)
        in_maps.append({"preds": p, "tgts": t})
    res = run_bass_kernel_spmd(nc, in_maps, list(range(NCORES))).results
    tot = np.zeros(3, np.float64)
    for r in res:
        tot += r["out"][0, 0:3].astype(np.float64)
    denom = float(B * STAGES)
    loss = (2.0 * tot[0] + 0.2 * tot[1] + 2.0 * tot[2]) / denom + float(seg_loss)
    return np.float32(loss)


if __name__ == "__main__":
    build()
    print("build OK")
